# revision 13
# baseline (speedup 1.0000x reference)
"""AttentionCropLayer Trainium2 kernel.

Per sample b: offsets (w,h) = floor(clip(locs[b]*224, 44, 180) - 44); output
out[b] = images[b, :, w:w+88, h:h+88] * mask, with mask the fixed 88x88
sigmoid-profile outer product.

Strategy (pure data parallel, 8 cores x 16 samples):
  - host precomputes, per sample, the flat element offset w*224 + h (exact
    same f32 ops as the reference) and the constant mask replicated to 128
    partitions
  - device, per chunk of 8 samples (= 128 partitions, one (sample,channel)
    image block per partition): 8 dynamic-offset HWDGE DMAs, each reading 16
    contiguous 78KB runs (one per channel) starting at the crop origin.
    Because h <= 136 and the crop is 88 wide, crop element (i,k) sits at
    position i*224 + k of the run: a static strided AP, so one in-place DVE
    multiply by the mask compacts [*,224-strided 88x88] -> [*, 7744] while
    applying the mask. One contiguous 3.96MB DMA writes the chunk out.
"""

import sys

if "/opt/trn_rl_repo" not in sys.path:
    sys.path.insert(0, "/opt/trn_rl_repo")

import numpy as np

import concourse.bass as bass
import concourse.bacc as bacc
import concourse.mybir as mybir
from concourse import tile
from concourse.bass_utils import run_bass_kernel_spmd

TL = 44
CROP = 2 * TL          # 88
SCALE = 224.0
B, C, IN = 128, 16, 224
NCORES = 8
BPC = B // NCORES      # 16 samples per core
BLK = 8                # samples per chunk -> BLK*C = 128 partitions
NBLK = BPC // BLK      # 2 chunks per core
FREE = CROP * CROP     # 7744 compact crop elements
RAW = (CROP - 1) * IN + CROP  # 19576: contiguous run covering the crop
MAXOFF = IN - CROP     # 136
IMSZ = C * IN * IN     # elements per sample image
CHSZ = IN * IN         # elements per channel image
MAXEOFF = (BPC - 1) * IMSZ + MAXOFF * IN + MAXOFF

_nc_cache = {}


def _build_nc():
    nc = bacc.Bacc(None, dynamic_dma_scratch_size=2048)
    images = nc.declare_dram_parameter(
        "images", [BPC, C, IN, IN], mybir.dt.float32, isOutput=False
    )
    offs = nc.declare_dram_parameter(
        "offs", [1, BPC], mybir.dt.int32, isOutput=False
    )
    mask128 = nc.declare_dram_parameter(
        "mask128", [BLK * C, FREE], mybir.dt.float32, isOutput=False
    )
    out = nc.declare_dram_parameter(
        "out", [BPC, C, CROP, CROP], mybir.dt.float32, isOutput=True
    )

    with tile.TileContext(nc) as tc:
        with (
            tc.tile_pool(name="const", bufs=1) as cpool,
            tc.tile_pool(name="work", bufs=2) as wpool,
        ):
            offs_sb = cpool.tile([1, BPC], mybir.dt.int32)
            nc.sync.dma_start(out=offs_sb[:], in_=offs[:])
            mask_sb = cpool.tile([BLK * C, FREE], mybir.dt.float32)
            nc.scalar.dma_start(out=mask_sb[:], in_=mask128[:])

            o_reg = nc.sync.alloc_register("o_reg")
            for blk in range(NBLK):
                t = wpool.tile([BLK * C, RAW], mybir.dt.float32, tag="blk")
                for j in range(BLK):
                    s = blk * BLK + j
                    nc.sync.reg_load(o_reg, offs_sb[0:1, s : s + 1])
                    ov = nc.sync.snap(o_reg, donate=True, min_val=0, max_val=MAXEOFF)
                    base = images[s, :, 0:CROP, 0:CROP]
                    src = bass.AP(
                        tensor=base.tensor,
                        offset=ov,
                        ap=[[CHSZ, C], [1, RAW]],
                        dep_tracking_offset=s * IMSZ,
                    )
                    nc.sync.dma_start(out=t[j * C : (j + 1) * C, :], in_=src)
                # in-place masked compaction: read (i,k) at free offset
                # i*224+k, write at i*88+k; the write offset never overtakes
                # the read offset, so the in-place op is safe
                t_ap = t[:]
                t_crop = bass.AP(
                    tensor=t_ap.tensor,
                    offset=t_ap.offset,
                    ap=[t_ap.ap[0], [IN, CROP], [1, CROP]],
                )
                nc.vector.tensor_tensor(
                    out=t[:, 0:FREE], in0=t_crop, in1=mask_sb[:],
                    op=mybir.AluOpType.mult,
                )
                out_view = out[blk * BLK : (blk + 1) * BLK].rearrange(
                    "b c i k -> (b c) (i k)"
                )
                nc.scalar.dma_start(out=out_view, in_=t[:, 0:FREE])
    nc.finalize()
    return nc


def _get_nc():
    if "nc" not in _nc_cache:
        _nc_cache["nc"] = _build_nc()
    return _nc_cache["nc"]


def _host_offsets(locs):
    locs = np.asarray(locs, dtype=np.float32)
    t = np.clip(locs * np.float32(SCALE), np.float32(TL), np.float32(IN - TL))
    return np.floor(t - np.float32(TL)).astype(np.int32)  # [B, 2] (w, h)


def _host_mask():
    rr = np.arange(CROP, dtype=np.float32)
    sig = lambda z: (1.0 / (1.0 + np.exp(-10.0 * z, dtype=np.float32))).astype(
        np.float32
    )
    prof = sig(rr) - sig(rr - np.float32(CROP))
    mask = np.outer(prof, prof).astype(np.float32).reshape(-1)
    return np.ascontiguousarray(np.tile(mask[None, :], (BLK * C, 1)))


def make_in_maps(images, locs):
    images = np.asarray(images, dtype=np.float32)
    off = _host_offsets(locs)  # [B, 2] (w, h)
    s_idx = np.arange(BPC, dtype=np.int64)
    mask128 = _host_mask()
    in_maps = []
    for i in range(NCORES):
        sl = slice(i * BPC, (i + 1) * BPC)
        osh = off[sl].astype(np.int64)
        eoff = (s_idx * IMSZ + osh[:, 0] * IN + osh[:, 1]).astype(np.int32)
        in_maps.append(
            {
                "images": np.ascontiguousarray(images[sl]),
                "offs": np.ascontiguousarray(eoff.reshape(1, -1)),
                "mask128": mask128,
            }
        )
    return in_maps


def run(images, locs, trace=False, **kwargs):
    nc = _get_nc()
    in_maps = make_in_maps(images, locs)
    res = run_bass_kernel_spmd(
        nc, in_maps, core_ids=list(range(NCORES)), trace=trace, **kwargs
    )
    outs = [np.asarray(res.results[i]["out"]) for i in range(NCORES)]
    full = np.concatenate(outs, axis=0).astype(np.float32)
    return full, res


def kernel(images, locs):
    full, _ = run(images, locs, trace=False)
    return full


# revision 15
# speedup vs baseline: 1.0363x; 1.0363x over previous
"""AttentionCropLayer Trainium2 kernel.

Per sample b: offsets (w,h) = floor(clip(locs[b]*224, 44, 180) - 44); output
out[b] = images[b, :, w:w+88, h:h+88] * mask, with mask the fixed 88x88
sigmoid-profile outer product.

Strategy (pure data parallel, 8 cores x 16 samples):
  - host precomputes, per sample, the flat element offset w*224 + h (exact
    same f32 ops as the reference) and the constant mask replicated to 128
    partitions
  - device, per chunk of 8 samples (= 128 partitions, one (sample,channel)
    image block per partition): 8 dynamic-offset HWDGE DMAs, each reading 16
    contiguous 78KB runs (one per channel) starting at the crop origin.
    Because h <= 136 and the crop is 88 wide, crop element (i,k) sits at
    position i*224 + k of the run: a static strided AP, so one in-place DVE
    multiply by the mask compacts [*,224-strided 88x88] -> [*, 7744] while
    applying the mask. One contiguous 3.96MB DMA writes the chunk out.
"""

import sys

if "/opt/trn_rl_repo" not in sys.path:
    sys.path.insert(0, "/opt/trn_rl_repo")

import numpy as np

import concourse.bass as bass
import concourse.bacc as bacc
import concourse.mybir as mybir
from concourse import tile
from concourse.bass_utils import run_bass_kernel_spmd

TL = 44
CROP = 2 * TL          # 88
SCALE = 224.0
B, C, IN = 128, 16, 224
NCORES = 8
BPC = B // NCORES      # 16 samples per core
BLK = 8                # samples per chunk -> BLK*C = 128 partitions
NBLK = BPC // BLK      # 2 chunks per core
FREE = CROP * CROP     # 7744 compact crop elements
G = 4                  # crop rows per read descriptor (must divide 88)
NRUN = CROP // G       # descriptors per (sample, channel) block
RUN = (G - 1) * IN + CROP  # elements per descriptor (row-wrap trick)
RAW = NRUN * RUN       # elements per partition as loaded
MAXOFF = IN - CROP     # 136
IMSZ = C * IN * IN     # elements per sample image
CHSZ = IN * IN         # elements per channel image
MAXEOFF = (BPC - 1) * IMSZ + MAXOFF * IN + MAXOFF

_nc_cache = {}


def _build_nc():
    nc = bacc.Bacc(None, dynamic_dma_scratch_size=2048)
    images = nc.declare_dram_parameter(
        "images", [BPC, C, IN, IN], mybir.dt.float32, isOutput=False
    )
    offs = nc.declare_dram_parameter(
        "offs", [1, BPC], mybir.dt.int32, isOutput=False
    )
    mask128 = nc.declare_dram_parameter(
        "mask128", [BLK * C, FREE], mybir.dt.float32, isOutput=False
    )
    out = nc.declare_dram_parameter(
        "out", [BPC, C, CROP, CROP], mybir.dt.float32, isOutput=True
    )

    with tile.TileContext(nc) as tc:
        with (
            tc.tile_pool(name="const", bufs=1) as cpool,
            tc.tile_pool(name="work", bufs=2) as wpool,
        ):
            offs_sb = cpool.tile([1, BPC], mybir.dt.int32)
            nc.sync.dma_start(out=offs_sb[:], in_=offs[:])
            mask_sb = cpool.tile([BLK * C, FREE], mybir.dt.float32)
            nc.scalar.dma_start(out=mask_sb[:], in_=mask128[:])

            o_reg = nc.sync.alloc_register("o_reg")
            for blk in range(NBLK):
                t = wpool.tile([BLK * C, RAW], mybir.dt.float32, tag="blk")
                for j in range(BLK):
                    s = blk * BLK + j
                    nc.sync.reg_load(o_reg, offs_sb[0:1, s : s + 1])
                    ov = nc.sync.snap(o_reg, donate=True, min_val=0, max_val=MAXEOFF)
                    base = images[s, :, 0:CROP, 0:CROP]
                    src = bass.AP(
                        tensor=base.tensor,
                        offset=ov,
                        ap=[[CHSZ, C], [G * IN, NRUN], [1, RUN]],
                        dep_tracking_offset=s * IMSZ,
                    )
                    nc.sync.dma_start(out=t[j * C : (j + 1) * C, :], in_=src)
                # in-place masked compaction: crop element (i=G*q+r, k) sits
                # at free offset q*RUN + r*224 + k; write offset (88i+k)
                # never overtakes the read offset, so in-place is safe
                t_ap = t[:]
                t_crop = bass.AP(
                    tensor=t_ap.tensor,
                    offset=t_ap.offset,
                    ap=[t_ap.ap[0], [RUN, NRUN], [IN, G], [1, CROP]],
                )
                nc.vector.tensor_tensor(
                    out=t[:, 0:FREE], in0=t_crop, in1=mask_sb[:],
                    op=mybir.AluOpType.mult,
                )
                out_view = out[blk * BLK : (blk + 1) * BLK].rearrange(
                    "b c i k -> (b c) (i k)"
                )
                nc.scalar.dma_start(out=out_view, in_=t[:, 0:FREE])
    nc.finalize()
    return nc


def _get_nc():
    if "nc" not in _nc_cache:
        _nc_cache["nc"] = _build_nc()
    return _nc_cache["nc"]


def _host_offsets(locs):
    locs = np.asarray(locs, dtype=np.float32)
    t = np.clip(locs * np.float32(SCALE), np.float32(TL), np.float32(IN - TL))
    return np.floor(t - np.float32(TL)).astype(np.int32)  # [B, 2] (w, h)


def _host_mask():
    rr = np.arange(CROP, dtype=np.float32)
    sig = lambda z: (1.0 / (1.0 + np.exp(-10.0 * z, dtype=np.float32))).astype(
        np.float32
    )
    prof = sig(rr) - sig(rr - np.float32(CROP))
    mask = np.outer(prof, prof).astype(np.float32).reshape(-1)
    return np.ascontiguousarray(np.tile(mask[None, :], (BLK * C, 1)))


def make_in_maps(images, locs):
    images = np.asarray(images, dtype=np.float32)
    off = _host_offsets(locs)  # [B, 2] (w, h)
    s_idx = np.arange(BPC, dtype=np.int64)
    mask128 = _host_mask()
    in_maps = []
    for i in range(NCORES):
        sl = slice(i * BPC, (i + 1) * BPC)
        osh = off[sl].astype(np.int64)
        eoff = (s_idx * IMSZ + osh[:, 0] * IN + osh[:, 1]).astype(np.int32)
        in_maps.append(
            {
                "images": np.ascontiguousarray(images[sl]),
                "offs": np.ascontiguousarray(eoff.reshape(1, -1)),
                "mask128": mask128,
            }
        )
    return in_maps


def run(images, locs, trace=False, **kwargs):
    nc = _get_nc()
    in_maps = make_in_maps(images, locs)
    res = run_bass_kernel_spmd(
        nc, in_maps, core_ids=list(range(NCORES)), trace=trace, **kwargs
    )
    outs = [np.asarray(res.results[i]["out"]) for i in range(NCORES)]
    full = np.concatenate(outs, axis=0).astype(np.float32)
    return full, res


def kernel(images, locs):
    full, _ = run(images, locs, trace=False)
    return full


# revision 18
# speedup vs baseline: 1.6613x; 1.6031x over previous
"""AttentionCropLayer Trainium2 kernel.

Per sample b: offsets (w,h) = floor(clip(locs[b]*224, 44, 180) - 44); output
out[b] = images[b, :, w:w+88, h:h+88] * mask, with mask the fixed 88x88
sigmoid-profile outer product.

Strategy (pure data parallel, 8 cores x 16 samples):
  - host precomputes, per sample, the flat element offset w*224 + h (exact
    same f32 ops as the reference) and the constant mask replicated to 128
    partitions
  - device, per chunk of 8 samples (= 128 partitions, one (sample,channel)
    image block per partition): 8 dynamic-offset HWDGE DMAs, each reading 16
    contiguous 78KB runs (one per channel) starting at the crop origin.
    Because h <= 136 and the crop is 88 wide, crop element (i,k) sits at
    position i*224 + k of the run: a static strided AP, so one in-place DVE
    multiply by the mask compacts [*,224-strided 88x88] -> [*, 7744] while
    applying the mask. One contiguous 3.96MB DMA writes the chunk out.
"""

import sys

if "/opt/trn_rl_repo" not in sys.path:
    sys.path.insert(0, "/opt/trn_rl_repo")

import numpy as np

import concourse.bass as bass
import concourse.bacc as bacc
import concourse.mybir as mybir
from concourse import tile
from concourse.bass_utils import run_bass_kernel_spmd

TL = 44
CROP = 2 * TL          # 88
SCALE = 224.0
B, C, IN = 128, 16, 224
NCORES = 8
BPC = B // NCORES      # 16 samples per core
BLK = 8                # samples per chunk -> BLK*C = 128 partitions
NBLK = BPC // BLK      # 2 chunks per core
FREE = CROP * CROP     # 7744 compact crop elements
G = 4                  # crop rows per read descriptor (must divide 88)
NRUN = CROP // G       # descriptors per (sample, channel) block
RUN = (G - 1) * IN + CROP  # elements per descriptor (row-wrap trick)
RAW = NRUN * RUN       # elements per partition as loaded
MAXOFF = IN - CROP     # 136
IMSZ = C * IN * IN     # elements per sample image
CHSZ = IN * IN         # elements per channel image
MAXEOFF = (BPC - 1) * IMSZ + MAXOFF * IN + MAXOFF

_nc_cache = {}


def _build_nc():
    nc = bacc.Bacc(None, dynamic_dma_scratch_size=2048)
    images = nc.declare_dram_parameter(
        "images", [BPC, C, IN, IN], mybir.dt.float32, isOutput=False
    )
    offs = nc.declare_dram_parameter(
        "offs", [1, BPC], mybir.dt.int32, isOutput=False
    )
    mask128 = nc.declare_dram_parameter(
        "mask128", [BLK * C, FREE], mybir.dt.float32, isOutput=False
    )
    out = nc.declare_dram_parameter(
        "out", [BPC, C, CROP, CROP], mybir.dt.float32, isOutput=True
    )

    with tile.TileContext(nc) as tc:
        with (
            tc.tile_pool(name="const", bufs=1) as cpool,
            tc.tile_pool(name="work", bufs=2) as wpool,
        ):
            offs_sb = cpool.tile([1, BPC], mybir.dt.int32)
            nc.sync.dma_start(out=offs_sb[:], in_=offs[:])
            mask_sb = cpool.tile([BLK * C, FREE], mybir.dt.float32)
            nc.gpsimd.dma_start(out=mask_sb[:], in_=mask128[:])

            o_reg = nc.sync.alloc_register("o_reg")
            o_reg2 = nc.scalar.alloc_register("o_reg2")
            for blk in range(NBLK):
                t = wpool.tile([BLK * C, RAW], mybir.dt.float32, tag="blk")
                for j in range(BLK):
                    s = blk * BLK + j
                    # alternate the two HWDGE rings so each SDMA engine has
                    # two independent packet streams in flight
                    eng_, reg_ = (
                        (nc.sync, o_reg) if j % 2 == 0 else (nc.scalar, o_reg2)
                    )
                    eng_.reg_load(reg_, offs_sb[0:1, s : s + 1])
                    ov = eng_.snap(reg_, donate=True, min_val=0, max_val=MAXEOFF)
                    base = images[s, :, 0:CROP, 0:CROP]
                    src = bass.AP(
                        tensor=base.tensor,
                        offset=ov,
                        ap=[[CHSZ, C], [G * IN, NRUN], [1, RUN]],
                        dep_tracking_offset=s * IMSZ,
                    )
                    eng_.dma_start(out=t[j * C : (j + 1) * C, :], in_=src)
                # in-place masked compaction: crop element (i=G*q+r, k) sits
                # at free offset q*RUN + r*224 + k; write offset (88i+k)
                # never overtakes the read offset, so in-place is safe
                t_ap = t[:]
                t_crop = bass.AP(
                    tensor=t_ap.tensor,
                    offset=t_ap.offset,
                    ap=[t_ap.ap[0], [RUN, NRUN], [IN, G], [1, CROP]],
                )
                nc.vector.tensor_tensor(
                    out=t[:, 0:FREE], in0=t_crop, in1=mask_sb[:],
                    op=mybir.AluOpType.mult,
                )
                out_view = out[blk * BLK : (blk + 1) * BLK].rearrange(
                    "b c i k -> (b c) (i k)"
                )
                nc.gpsimd.dma_start(out=out_view, in_=t[:, 0:FREE])
    nc.finalize()
    return nc


def _get_nc():
    if "nc" not in _nc_cache:
        _nc_cache["nc"] = _build_nc()
    return _nc_cache["nc"]


def _host_offsets(locs):
    locs = np.asarray(locs, dtype=np.float32)
    t = np.clip(locs * np.float32(SCALE), np.float32(TL), np.float32(IN - TL))
    return np.floor(t - np.float32(TL)).astype(np.int32)  # [B, 2] (w, h)


def _host_mask():
    rr = np.arange(CROP, dtype=np.float32)
    sig = lambda z: (1.0 / (1.0 + np.exp(-10.0 * z, dtype=np.float32))).astype(
        np.float32
    )
    prof = sig(rr) - sig(rr - np.float32(CROP))
    mask = np.outer(prof, prof).astype(np.float32).reshape(-1)
    return np.ascontiguousarray(np.tile(mask[None, :], (BLK * C, 1)))


def make_in_maps(images, locs):
    images = np.asarray(images, dtype=np.float32)
    off = _host_offsets(locs)  # [B, 2] (w, h)
    s_idx = np.arange(BPC, dtype=np.int64)
    mask128 = _host_mask()
    in_maps = []
    for i in range(NCORES):
        sl = slice(i * BPC, (i + 1) * BPC)
        osh = off[sl].astype(np.int64)
        eoff = (s_idx * IMSZ + osh[:, 0] * IN + osh[:, 1]).astype(np.int32)
        in_maps.append(
            {
                "images": np.ascontiguousarray(images[sl]),
                "offs": np.ascontiguousarray(eoff.reshape(1, -1)),
                "mask128": mask128,
            }
        )
    return in_maps


def run(images, locs, trace=False, **kwargs):
    nc = _get_nc()
    in_maps = make_in_maps(images, locs)
    res = run_bass_kernel_spmd(
        nc, in_maps, core_ids=list(range(NCORES)), trace=trace, **kwargs
    )
    outs = [np.asarray(res.results[i]["out"]) for i in range(NCORES)]
    full = np.concatenate(outs, axis=0).astype(np.float32)
    return full, res


def kernel(images, locs):
    full, _ = run(images, locs, trace=False)
    return full
